# revision 41
# baseline (speedup 1.0000x reference)
"""CACE GNN message-passing kernel for 8 trn2 NeuronCores.

Node-parallel sharding: 625 nodes/core, 40 groups of 16 nodes. Edges sorted by
receiver; each group's edges fill 2 matmul slots of 128 edges (PSUM
accumulation). Per slot one fp16 matmul (lhsT = onehot x radial/4 [128e, 128],
rhs = angular x encoded [128e, 180]) scatters rank-1 edge tensors into the
group's node bucket. A is stored plane-major [128p, 20m, 40g*9c] so the nu=2..4
symmetrization (fp16, batched across planes, split across DVE/Pool/ACT) runs on
contiguous 360-element runs. Outputs are rescaled/transposed on the host.
"""
import math
import functools
import numpy as np

# ---------------- problem constants (hardcoded; must match reference) -------
N_NODES, N_EDGES = 5000, 50000
N_RBF, MAX_L = 8, 3
CUTOFF = 5.5
EPS = 1e-9
ZS = [1, 6, 7, 8]
N_CORES = 8
PER = N_NODES // N_CORES          # 625 nodes per core
NG = 40                           # 16-node groups per core
GN = 16                           # nodes per group
NS = 2 * NG                       # matmul slots (128 edges each)
P = 128
NQ = GN * N_RBF                   # 128 = matmul out partitions
NM = 20
NC9 = 9
NF = 11
W = NG * NC9                      # 360 = flat (group, channel) width
SCALE = 0.25                      # A is computed as A/4 (fp16 headroom)
SQ2C = math.sqrt(2.0 / CUTOFF)
F_UNSCALE = np.array([4.0] + [16.0] * 3 + [64.0] * 2 + [256.0] * 5,
                     np.float32)


# ---------------- device kernel build --------------------------------------
@functools.lru_cache(maxsize=2)
def _build_nc(debug=False):
    import concourse.bacc as bacc
    import concourse.mybir as mybir
    from concourse.tile import TileContext

    f32 = mybir.dt.float32
    f16 = mybir.dt.float16
    mul = mybir.AluOpType.mult
    add = mybir.AluOpType.add
    sub = mybir.AluOpType.subtract
    ACT = mybir.ActivationFunctionType

    nc = bacc.Bacc("TRN2", target_bir_lowering=False, debug=False,
                   num_devices=N_CORES)
    pos_d = nc.dram_tensor("pos", [P, NS * 6], f32, kind="ExternalInput")
    emb_d = nc.dram_tensor("emb", [P, NS * 6], f16, kind="ExternalInput")
    oh_d = nc.dram_tensor("oh", [P, NS * NQ], f16, kind="ExternalInput")
    out_d = nc.dram_tensor("out", [P, NF * W], f16, kind="ExternalOutput")
    dbg = {}
    if debug:
        dbg["A"] = nc.dram_tensor("dbg_A", [P, NM * W], f16,
                                  kind="ExternalOutput")

    with TileContext(nc) as tc:
        with (
            tc.tile_pool(name="keep", bufs=1) as kp,
            tc.tile_pool(name="psum", bufs=8, space="PSUM") as pp,
        ):
            ep_cm = tc.tile_pool(name="edge", bufs=1)
            ep = ep_cm.__enter__()
            pos = ep.tile([P, NS * 6], f32)
            emb = ep.tile([P, NS * 6], f16)
            oh = ep.tile([P, NS * NQ], f16)
            nc.sync.dma_start(out=pos[:, :], in_=pos_d[:, :])
            nc.sync.dma_start(out=emb[:, :], in_=emb_d[:, :])
            nc.sync.dma_start(out=oh[:, :], in_=oh_d[:, :])
            pv = pos[:, :].rearrange("p (s t) -> p s t", t=6)
            emv = emb[:, :].rearrange("p (s t) -> p s t", t=6)

            V, G, S = nc.vector, nc.gpsimd, nc.scalar

            # --- geometry (fp32, DVE) ---
            d = ep.tile([P, NS * 3], f32)
            dv = d[:, :].rearrange("p (s t) -> p s t", t=3)
            V.tensor_tensor(out=dv, in0=pv[:, :, 3:6], in1=pv[:, :, 0:3],
                            op=sub)
            dsq = ep.tile([P, NS * 3], f32)
            dsv = dsq[:, :].rearrange("p (s t) -> p s t", t=3)
            V.tensor_tensor(out=dsv, in0=dv, in1=dv, op=mul)
            l2 = ep.tile([P, NS], f32)
            V.tensor_reduce(out=l2[:, :], in_=dsv, axis=mybir.AxisListType.X,
                            op=add)
            ln = ep.tile([P, NS], f32)
            S.activation(out=ln[:, :], in_=l2[:, :], func=ACT.Sqrt)
            le = ep.tile([P, NS], f32)
            V.tensor_scalar_add(le[:, :], ln[:, :], EPS)
            rinv = ep.tile([P, NS], f32)
            V.reciprocal(out=rinv[:, :], in_=le[:, :])
            unit = ep.tile([P, NS * 3], f32)
            uv = unit[:, :].rearrange("p (s t) -> p s t", t=3)
            V.tensor_tensor(
                out=uv, in0=dv,
                in1=rinv[:, :].unsqueeze(2).to_broadcast([P, NS, 3]), op=mul)
            # unit replicated over 9 encoded channels (for recursive rhs)
            u9 = ep.tile([P, NS * 3 * NC9], f16)
            u9v = u9[:, :].rearrange("p (s a c) -> p s a c", a=3, c=NC9)
            S.copy(out=u9v[:, 0:NS // 2],
                   in_=uv[:, 0:NS // 2].unsqueeze(3).to_broadcast(
                       [P, NS // 2, 3, NC9]))
            S.copy(out=u9v[:, NS // 2:],
                   in_=uv[:, NS // 2:].unsqueeze(3).to_broadcast(
                       [P, NS // 2, 3, NC9]))

            rhs = ep.tile([P, NS * NM * NC9], f16)
            rv = rhs[:, :].rearrange("p (s m c) -> p s m c", m=NM, c=NC9)
            V.tensor_tensor(
                out=rv[:, :, 0, :].rearrange("p s (a b) -> p s a b", a=3, b=3),
                in0=emv[:, :, 0:3].unsqueeze(3).to_broadcast([P, NS, 3, 3]),
                in1=emv[:, :, 3:6].unsqueeze(2).to_broadcast([P, NS, 3, 3]),
                op=mul)
            # --- radial: sin(n*pi*l/C) via Chebyshev recurrence on DVE ---
            lc = ep.tile([P, NS], f32)
            V.tensor_scalar_min(lc[:, :], ln[:, :], CUTOFF)
            th = ep.tile([P, NS], f32)
            V.tensor_scalar_mul(th[:, :], lc[:, :], math.pi / CUTOFF)
            hh = ep.tile([P, NS], f32)
            V.tensor_scalar_mul(hh[:, :], lc[:, :], math.pi / (2.0 * CUTOFF))
            sh = ep.tile([P, NS], f32)
            S.activation(out=sh[:, :], in_=hh[:, :], func=ACT.Sin)
            shq = ep.tile([P, NS], f32)
            S.activation(out=shq[:, :], in_=sh[:, :], func=ACT.Square)
            c2 = ep.tile([P, NS], f32)
            nc.vector.tensor_scalar(c2[:, :], shq[:, :], -4.0, 2.0, mul, add)
            sinr = ep.tile([P, NS * N_RBF], f32)
            sv = sinr[:, :].rearrange("p (s r) -> p s r", r=N_RBF)
            S.activation(out=sv[:, :, 0], in_=th[:, :], func=ACT.Sin)
            V.tensor_tensor(out=sv[:, :, 1], in0=c2[:, :], in1=sv[:, :, 0],
                            op=mul)
            for n in range(2, N_RBF):
                tn = ep.tile([P, NS], f32, tag=f"cheb{n % 2}")
                V.tensor_tensor(out=tn[:, :], in0=c2[:, :],
                                in1=sv[:, :, n - 1], op=mul)
                V.tensor_tensor(out=sv[:, :, n], in0=tn[:, :],
                                in1=sv[:, :, n - 2], op=sub)
            # cutoff polynomial fc = 1 - 28u^6 + 48u^7 - 21u^8
            uu = ep.tile([P, NS], f32)
            V.tensor_scalar_mul(uu[:, :], lc[:, :], 1.0 / CUTOFF)
            u2 = ep.tile([P, NS], f32)
            S.activation(out=u2[:, :], in_=uu[:, :], func=ACT.Square)
            u3 = ep.tile([P, NS], f32)
            V.tensor_tensor(out=u3[:, :], in0=u2[:, :], in1=uu[:, :], op=mul)
            u6 = ep.tile([P, NS], f32)
            S.activation(out=u6[:, :], in_=u3[:, :], func=ACT.Square)
            t1 = ep.tile([P, NS], f32)
            nc.vector.tensor_scalar(t1[:, :], uu[:, :], -21.0, 48.0, mul, add)
            t2 = ep.tile([P, NS], f32)
            V.tensor_tensor(out=t2[:, :], in0=t1[:, :], in1=uu[:, :], op=mul)
            t3 = ep.tile([P, NS], f32)
            V.tensor_scalar_add(t3[:, :], t2[:, :], -28.0)
            fcv = ep.tile([P, NS], f32)
            V.tensor_tensor(out=fcv[:, :], in0=u6[:, :], in1=t3[:, :], op=mul)
            w1 = ep.tile([P, NS], f32)
            nc.vector.tensor_scalar(w1[:, :], fcv[:, :], SQ2C * SCALE,
                                    SQ2C * SCALE, mul, add)
            wfac = ep.tile([P, NS], f32)
            V.tensor_tensor(out=wfac[:, :], in0=w1[:, :], in1=rinv[:, :],
                            op=mul)
            rad = ep.tile([P, NS * N_RBF], f16)
            rdv = rad[:, :].rearrange("p (s r) -> p s r", r=N_RBF)
            V.tensor_tensor(
                out=rdv, in0=sinr[:, :].rearrange("p (s r) -> p s r", r=N_RBF),
                in1=wfac[:, :].unsqueeze(2).to_broadcast([P, NS, N_RBF]),
                op=mul)

            # --- encoded -> rhs[m=0]; recursive rhs build (fp16 2x) ---
            rhs = ep.tile([P, NS * NM * NC9], f16)
            rv = rhs[:, :].rearrange("p (s m c) -> p s m c", m=NM, c=NC9)
            lhsT = ep.tile([P, NS * NQ], f16)
            lv = lhsT[:, :].rearrange("p (s n r) -> p s n r", n=GN, r=N_RBF)
            ohv = oh[:, :].rearrange("p (s n r) -> p s n r", n=GN, r=N_RBF)
            NH = NS // 2
            for h0 in (0, NH):
                hs = slice(h0, h0 + NH)
                V.tensor_tensor(
                    out=lv[:, hs], in0=ohv[:, hs],
                    in1=rdv[:, hs].unsqueeze(2).to_broadcast(
                        [P, NH, GN, N_RBF]), op=mul)
                V.tensor_tensor(
                    out=rv[:, hs, 1:4, :], in0=u9v[:, hs],
                    in1=rv[:, hs, 0:1, :].to_broadcast([P, NH, 3, NC9]),
                    op=mul)
                for (o0, o1, a, i0, i1) in [(4, 7, 0, 1, 4), (7, 9, 1, 2, 4),
                                            (9, 10, 2, 3, 4), (10, 16, 0, 4, 10),
                                            (16, 19, 1, 7, 10),
                                            (19, 20, 2, 9, 10)]:
                    V.tensor_tensor(
                        out=rv[:, hs, o0:o1, :],
                        in0=u9v[:, hs, a:a + 1, :].to_broadcast(
                            [P, NH, o1 - o0, NC9]),
                        in1=rv[:, hs, i0:i1, :], op=mul)

            # --- scatter matmuls -> A plane-major [P, 20m, 40g, 9c] ---
            A = kp.tile([P, NM * W], f16)
            Am = A[:, :].rearrange("p (m g c) -> p m g c", m=NM, g=NG, c=NC9)
            lvf = lhsT[:, :].rearrange("p (s q) -> p s q", q=NQ)
            rvf = rhs[:, :].rearrange("p (s f) -> p s f", f=NM * NC9)
            for gp in range(NG // 2):
                pt = pp.tile([P, 2 * NM * NC9], f32)
                for h in range(2):
                    g = gp * 2 + h
                    for s2_ in range(2):
                        nc.tensor.matmul(
                            out=pt[:, h * 180:(h + 1) * 180],
                            lhsT=lvf[:, 2 * g + s2_, :],
                            rhs=rvf[:, 2 * g + s2_, :],
                            start=(s2_ == 0), stop=(s2_ == 1))
                S.copy(out=Am[:, :, 2 * gp:2 * gp + 2, :],
                       in_=pt[:, :].rearrange("p (h m c) -> p m h c", h=2,
                                              m=NM, c=NC9))
            if debug:
                nc.sync.dma_start(out=dbg["A"][:, :], in_=A[:, :])

            # ---- symmetrization: flat [P, k*360] fp16 slabs ----
            ep_cm.__exit__(None, None, None)
            sy_cm = tc.tile_pool(name="sym", bufs=1)
            sy = sy_cm.__enter__()

            def mk(name, k):
                return sy.tile([P, k * W], f16, name=name, tag=name)

            def fl(t, k0, k1):
                return t[:, k0 * W:k1 * W]

            def v4(t, k, j):
                return t[:, :].rearrange("p (k j w) -> p k j w", k=k, j=j,
                                         w=W)

            Ap = lambda m0, m1: fl(A, m0, m1)
            Q9t = mk("q9", 6)
            TFt = mk("tf", 12)
            PPt = mk("pp", 54)
            zMt = mk("zm", 9)
            s1t = mk("s1", 9)
            s2t = mk("s2", 9)
            PUt = mk("pu", 9)
            ut = mk("u", 3)
            P2pt = mk("p2p", 18)
            P2t = mk("p2", 6)
            W3t = mk("w3", 12)
            w3s1t = mk("w3s1", 4)
            WSt = mk("ws", 18)
            wss1t = mk("wss1", 3)
            wss2t = mk("wss2", 3)
            FSt = mk("fs", NF)
            smt = mk("sm", 12)

            def tt(eng, o, a, b, op=mul):
                eng.tensor_tensor(out=o, in0=a, in1=b, op=op)

            def A4(m0, m1, h):
                return A[:, :].rearrange("p (m g c) -> p m g c", m=NM,
                                         g=NG, c=NC9)[:, m0:m1,
                                                      20 * h:20 * h + 20, :]

            def T4(t, k, k0, k1, h):
                return t[:, :].rearrange("p (k g c) -> p k g c", k=k,
                                         g=NG, c=NC9)[:, k0:k1,
                                                      20 * h:20 * h + 20, :]

            # --- S queue: TF gathers, Mdiag squares, row squares, Qs ---
            for (k0, m0, m1) in [(0, 11, 12), (1, 13, 15), (3, 16, 19),
                                 (6, 12, 13), (7, 14, 16), (9, 17, 20)]:
                S.copy(out=fl(TFt, k0, k0 + m1 - m0), in_=Ap(m0, m1))
            S.activation(out=fl(PPt, 18, 24), in_=Ap(10, 16),
                         func=ACT.Square)
            S.activation(out=fl(PPt, 36, 42), in_=fl(TFt, 0, 6),
                         func=ACT.Square)
            S.activation(out=fl(PPt, 48, 54), in_=fl(TFt, 6, 12),
                         func=ACT.Square)
            S.activation(out=fl(WSt, 0, 6), in_=Ap(4, 10), func=ACT.Square)
            S.activation(out=fl(W3t, 0, 3), in_=Ap(1, 4), func=ACT.Square)
            S.activation(out=Q9t[:, :], in_=Ap(4, 10), func=ACT.Square)

            # A-only blocks (full width; per-group-half splitting gives no
            # early start -- dependency tracking on A is tile-coarse)
            tt(V, fl(PPt, 0, 6), Ap(10, 16), Ap(4, 10))
            tt(V, fl(PUt, 0, 3), Ap(4, 7), Ap(1, 4))
            tt(V, fl(PUt, 3, 4), Ap(5, 6), Ap(1, 2))
            tt(V, fl(PUt, 4, 6), Ap(7, 9), Ap(2, 4))
            tt(V, fl(PUt, 6, 7), Ap(6, 7), Ap(1, 2))
            tt(V, fl(PUt, 7, 9), Ap(8, 10), Ap(2, 4))
            tt(V, fl(P2pt, 0, 3), Ap(10, 13), Ap(1, 4))
            for (row, ma, mb) in [(1, 11, 13), (2, 12, 14), (3, 13, 16),
                                  (4, 14, 17), (5, 15, 18)]:
                tt(V, fl(P2pt, 3 * row, 3 * row + 1), Ap(ma, ma + 1), Ap(1, 2))
                tt(V, fl(P2pt, 3 * row + 1, 3 * row + 3), Ap(mb, mb + 2),
                   Ap(2, 4))
            PU3 = v4(PUt, 3, 3)
            tt(V, ut[:, :], PU3[:, :, 0, :], PU3[:, :, 1, :], add)
            tt(V, ut[:, :], ut[:, :], PU3[:, :, 2, :], add)
            P23 = v4(P2pt, 6, 3)
            tt(V, P2t[:, :], P23[:, :, 0, :], P23[:, :, 1, :], add)
            tt(V, P2t[:, :], P2t[:, :], P23[:, :, 2, :], add)
            # trS3 products/sums (A and Qs only)
            tt(V, fl(smt, 0, 1), Ap(4, 5), fl(Q9t, 0, 1))
            tt(V, fl(smt, 1, 2), Ap(7, 8), fl(Q9t, 3, 4))
            tt(V, fl(smt, 2, 3), Ap(9, 10), fl(Q9t, 5, 6))
            tt(V, fl(smt, 0, 1), fl(smt, 0, 1), fl(smt, 1, 2), add)
            tt(V, fl(smt, 0, 1), fl(smt, 0, 1), fl(smt, 2, 3), add)
            tt(V, fl(smt, 3, 4), Ap(4, 5), Ap(7, 8), add)
            tt(V, fl(smt, 4, 5), Ap(4, 5), Ap(9, 10), add)
            tt(V, fl(smt, 5, 6), Ap(7, 8), Ap(9, 10), add)
            tt(V, fl(smt, 6, 8), fl(smt, 3, 5), fl(Q9t, 1, 3))
            tt(V, fl(smt, 8, 9), fl(smt, 5, 6), fl(Q9t, 4, 5))
            tt(V, fl(smt, 6, 7), fl(smt, 6, 7), fl(smt, 7, 8), add)
            tt(V, fl(smt, 6, 7), fl(smt, 6, 7), fl(smt, 8, 9), add)
            tt(V, fl(smt, 9, 10), Ap(5, 6), Ap(6, 7))
            tt(V, fl(smt, 9, 10), fl(smt, 9, 10), Ap(8, 9))

            # --- z/M complex (TX read straight from A[10:16]) ---
            TF3 = TFt[:, :].rearrange("p (j k w) -> p j k w", j=2, k=6, w=W)
            PP3 = PPt[:, :].rearrange("p (k j w) -> p k j w", k=9, j=6, w=W)
            tt(V, PP3[:, 1:3, :, :], TF3,
               Ap(4, 10).rearrange("p (k w) -> p k w", w=W).unsqueeze(1)
               .to_broadcast([P, 2, 6, W]))
            tt(V, fl(PPt, 24, 30), Ap(10, 16), fl(TFt, 0, 6))
            tt(V, fl(PPt, 30, 36), Ap(10, 16), fl(TFt, 6, 12))
            tt(V, fl(PPt, 42, 48), fl(TFt, 0, 6), fl(TFt, 6, 12))
            # weighted sum over ab: w={1,2,2,1,2,1}
            tt(V, s1t[:, :], PP3[:, :, 1, :], PP3[:, :, 2, :], add)
            tt(V, s1t[:, :], s1t[:, :], PP3[:, :, 4, :], add)
            tt(V, s2t[:, :], PP3[:, :, 0, :], PP3[:, :, 3, :], add)
            tt(V, s2t[:, :], s2t[:, :], PP3[:, :, 5, :], add)
            tt(V, s2t[:, :], s2t[:, :], s1t[:, :], add)
            tt(V, zMt[:, :], s2t[:, :], s1t[:, :], add)
            # F3 = Mxx+Myy+Mzz (zM rows {3,6,8})
            tt(V, fl(FSt, 3, 4), fl(zMt, 3, 4), fl(zMt, 6, 7), add)
            tt(V, fl(FSt, 3, 4), fl(FSt, 3, 4), fl(zMt, 8, 9), add)
            # nu3_2 products -> WS row2
            tt(V, fl(WSt, 12, 18), fl(zMt, 3, 9), Ap(4, 10))

            # --- S: dependent squares ---
            S.activation(out=fl(W3t, 3, 6), in_=ut[:, :], func=ACT.Square)
            S.activation(out=fl(W3t, 9, 12), in_=fl(zMt, 0, 3),
                         func=ACT.Square)
            S.activation(out=fl(WSt, 6, 12), in_=P2t[:, :], func=ACT.Square)
            S.activation(out=fl(smt, 10, 11), in_=fl(smt, 6, 7),
                         func=ACT.Copy, scale=3.0)
            S.activation(out=fl(smt, 11, 12), in_=fl(smt, 9, 10),
                         func=ACT.Copy, scale=6.0)

            # --- V: feature trees ---
            tt(V, fl(W3t, 6, 9), ut[:, :], fl(zMt, 0, 3))
            W33 = v4(W3t, 4, 3)
            tt(V, w3s1t[:, :], W33[:, :, 0, :], W33[:, :, 1, :], add)
            tt(V, fl(FSt, 1, 2), fl(w3s1t, 0, 1), fl(W3t, 2, 3), add)
            tt(V, fl(FSt, 6, 8), fl(w3s1t, 1, 3),
               W33[:, 1:3, 2, :], add)
            tt(V, fl(FSt, 10, 11), fl(w3s1t, 3, 4), fl(W3t, 11, 12), add)
            WS3 = v4(WSt, 3, 6)
            tt(V, wss1t[:, :], WS3[:, :, 1, :], WS3[:, :, 2, :], add)
            tt(V, wss1t[:, :], wss1t[:, :], WS3[:, :, 4, :], add)
            tt(V, wss2t[:, :], WS3[:, :, 0, :], WS3[:, :, 3, :], add)
            tt(V, wss2t[:, :], wss2t[:, :], WS3[:, :, 5, :], add)
            tt(V, wss2t[:, :], wss2t[:, :], wss1t[:, :], add)
            tt(V, fl(FSt, 2, 3), fl(wss2t, 0, 1), fl(wss1t, 0, 1), add)
            tt(V, fl(FSt, 8, 9), fl(wss2t, 1, 2), fl(wss1t, 1, 2), add)
            tt(V, fl(FSt, 5, 6), fl(wss2t, 2, 3), fl(wss1t, 2, 3), add)
            tt(V, fl(smt, 0, 1), fl(smt, 0, 1), fl(smt, 10, 11), add)
            tt(V, fl(FSt, 4, 5), fl(smt, 0, 1), fl(smt, 11, 12), add)
            S.copy(out=fl(FSt, 9, 10), in_=fl(FSt, 7, 8))
            S.copy(out=fl(FSt, 0, 1), in_=Ap(0, 1))

            # ---- dense output DMA (host transposes) ----
            nc.sync.dma_start(out=out_d[:, :], in_=FSt[:, :])
            sy_cm.__exit__(None, None, None)
    nc.compile()
    return nc


# ---------------- host side -------------------------------------------------
def _host_prep(inputs):
    pos = np.ascontiguousarray(inputs['positions'], np.float32)
    Wm = np.asarray(inputs['W_embed'], np.float32)
    an = np.asarray(inputs['atomic_numbers'])
    ei = np.asarray(inputs['edge_index'])
    zs = np.asarray(ZS, an.dtype)
    onehot = (an[:, None] == zs[None, :]).astype(np.float32)
    emb = (onehot @ Wm).astype(np.float16)
    send, recv = ei[0], ei[1]
    order = np.argsort(recv, kind='stable')
    send, recv = send[order], recv[order]
    counts = np.bincount(recv, minlength=N_NODES)
    starts = np.concatenate([[0], np.cumsum(counts)])
    in_maps = []
    for core in range(N_CORES):
        n0 = core * PER
        posb = np.zeros((P, NS, 6), np.float32)
        embb = np.zeros((P, NS, 6), np.float16)
        ohb = np.zeros((P, NS, NQ), np.float16)
        for g in range(NG):
            glo = n0 + GN * g
            ghi = min(glo + GN, n0 + PER)
            e0, e1 = starts[glo], starts[ghi]
            assert e1 - e0 <= 2 * P, f"group degree {e1-e0} > 256"
            for h in range(2):
                lo = e0 + h * P
                hi = min(e1, lo + P)
                if hi <= lo:
                    continue
                k = hi - lo
                s = 2 * g + h
                es, er = send[lo:hi], recv[lo:hi]
                posb[:k, s, 0:3] = pos[es]
                posb[:k, s, 3:6] = pos[er]
                embb[:k, s, 0:3] = emb[es]
                embb[:k, s, 3:6] = emb[er]
                rl = (er - glo)
                ohb[np.arange(k)[:, None], s,
                    (rl * N_RBF)[:, None] + np.arange(N_RBF)[None, :]] = 1.0
        in_maps.append({
            "pos": np.ascontiguousarray(posb.reshape(P, NS * 6)),
            "emb": np.ascontiguousarray(embb.reshape(P, NS * 6)),
            "oh": np.ascontiguousarray(ohb.reshape(P, NS * NQ)),
        })
    return in_maps


LAST = {}


def kernel(**inputs):
    import os
    from concourse.bass_utils import run_bass_kernel_spmd
    nc = _build_nc()
    in_maps = _host_prep(inputs)
    trace = bool(int(os.environ.get("KTRACE", "0")))
    res = run_bass_kernel_spmd(nc, in_maps, core_ids=list(range(N_CORES)),
                               trace=trace)
    LAST['res'] = res
    out = np.zeros((N_NODES, N_RBF, NF, NC9), np.float32)
    for core in range(N_CORES):
        # [128=(16n,8r), 11f*40g*9c] -> [g*16+n, r, f, c]
        slab = res.results[core]["out"].astype(np.float32).reshape(
            GN, N_RBF, NF, NG, NC9)
        slab = slab.transpose(3, 0, 1, 2, 4).reshape(NG * GN, N_RBF, NF, NC9)
        out[core * PER:(core + 1) * PER] = slab[:PER]
    out *= F_UNSCALE[None, None, :, None]
    return out
